# revision 11
# baseline (speedup 1.0000x reference)
"""Trainium2 Bass kernel for nn_LinearSEM.

Reference computes: z = solve_triangular(I - strict_lower(tril(w*mask)), (x*diag)^T).T
Algebraic reformulation: z = x @ W_eff with W_eff = diag(d) @ inv(I-L)^T —
the tiny 128x128 unit-lower-triangular inverse is computed on host in float64
(forward substitution, exact structure, no pivoting noise), and the device
kernel becomes a pure streaming GEMM, which is HBM-bandwidth-bound.

Sharding: data-parallel over batch across 8 cores. Each core receives its
x-shard pre-transposed ([128 vars, 65536 batch]) so the contraction dim (vars)
lands on SBUF partitions; W_eff is the PE-stationary operand (loaded once) and
x^T streams through 512 columns per matmul producing z^T, which is stored
contiguously and un-transposed on host.
"""

import numpy as np

NUM_VARS = 128
BATCH = 524288
N_CORES = 8
SHARD = BATCH // N_CORES  # 65536
DMA_TILE = 2048           # fp32 batch-cols per in-DMA tile: 128p x 8KB = 1 MiB/transfer
MM_N = 512                # max fp32 moving free dim per matmul


def _w_eff(weight: np.ndarray, mask: np.ndarray) -> np.ndarray:
    n = NUM_VARS
    wl = np.tril(weight.astype(np.float64) * mask.astype(np.float64))
    d = np.diag(wl).copy()
    L = wl - np.diag(d)
    # X = inv(I - L) by forward substitution in float64: X[i,:] = e_i + L[i,:i] @ X[:i,:]
    X = np.eye(n, dtype=np.float64)
    for i in range(1, n):
        X[i, :] += L[i, :i] @ X[:i, :]
    w_eff = d[:, None] * X.T
    return np.ascontiguousarray(w_eff.astype(np.float32))


def _build_bass(
    dma_tile=DMA_TILE,
    mm_n=MM_N,
    out_chunk=512,    # cols per output z tile / out-DMA; default = dma_tile
    xbufs=5,
    zbufs=12,
    pbufs=8,
    do_mm=True,
    do_copy=True,
    do_out=True,
    copy_engines="v",   # 'a'=ACT only, 'v'=DVE only, 'av'=alternate
    copy_split=False,   # split each PSUM->SBUF copy in half across ACT+DVE
    out_on_act=True,    # issue out-DMAs on the ACT HWDGE queue (separate FIFO)
    reps=1,             # repeat the whole sweep (for slope-based HW timing)
):
    import concourse.bacc as bacc
    import concourse.mybir as mybir
    from concourse.tile import TileContext

    if out_chunk is None:
        out_chunk = dma_tile
    assert dma_tile % out_chunk == 0 and out_chunk % mm_n == 0

    nc = bacc.Bacc(None, target_bir_lowering=False)
    xt = nc.dram_tensor("xt", [NUM_VARS, SHARD], mybir.dt.float32, kind="ExternalInput")
    w = nc.dram_tensor("w", [NUM_VARS, NUM_VARS], mybir.dt.float32, kind="ExternalInput")
    zt = nc.dram_tensor("zt", [NUM_VARS, SHARD], mybir.dt.float32, kind="ExternalOutput")

    with TileContext(nc) as tc:
        with (
            tc.tile_pool(name="wp", bufs=1) as wp,
            tc.tile_pool(name="xp", bufs=xbufs) as xp,
            tc.tile_pool(name="zp", bufs=zbufs) as zp,
            tc.tile_pool(name="pp", bufs=pbufs, space="PSUM") as pp,
        ):
            w_sb = wp.tile([NUM_VARS, NUM_VARS], mybir.dt.float32)
            nc.sync.dma_start(w_sb[:], w[:])
            nmm = 0
            for t in range(reps * (SHARD // dma_tile)):
                t = t % (SHARD // dma_tile)
                x_sb = xp.tile([NUM_VARS, dma_tile], mybir.dt.float32)
                nc.sync.dma_start(x_sb[:], xt[:, t * dma_tile:(t + 1) * dma_tile])
                for c in range(dma_tile // out_chunk):
                    z_sb = zp.tile([NUM_VARS, out_chunk], mybir.dt.float32)
                    for k in range(out_chunk // mm_n):
                        xsl = slice(c * out_chunk + k * mm_n,
                                    c * out_chunk + (k + 1) * mm_n)
                        zsl = slice(k * mm_n, (k + 1) * mm_n)
                        if do_mm:
                            ps = pp.tile([NUM_VARS, mm_n], mybir.dt.float32)
                            nc.tensor.matmul(
                                ps[:], w_sb[:], x_sb[:, xsl], start=True, stop=True,
                            )
                        if do_mm and do_copy:
                            if copy_split:
                                h = mm_n // 2
                                nc.scalar.copy(
                                    z_sb[:, zsl.start:zsl.start + h], ps[:, :h])
                                nc.vector.tensor_copy(
                                    z_sb[:, zsl.start + h:zsl.stop], ps[:, h:])
                            else:
                                eng = copy_engines[nmm % len(copy_engines)]
                                if eng == "a":
                                    nc.scalar.copy(z_sb[:, zsl], ps[:])
                                else:
                                    nc.vector.tensor_copy(z_sb[:, zsl], ps[:])
                            nmm += 1
                        elif do_out:
                            # plumb a dep so the out DMA still waits on something
                            nc.vector.tensor_copy(z_sb[:, zsl.start:zsl.start + 1],
                                                  x_sb[:, xsl.start:xsl.start + 1])
                    if do_out:
                        out_eng = nc.scalar if out_on_act else nc.sync
                        out_eng.dma_start(
                            zt[:, t * dma_tile + c * out_chunk:
                                  t * dma_tile + (c + 1) * out_chunk],
                            z_sb[:],
                        )
    nc.compile()
    return nc


_CACHE = {}


def kernel(x, weight, mask):
    from concourse.bass_utils import run_bass_kernel_spmd

    x = np.asarray(x, dtype=np.float32)
    weight = np.asarray(weight, dtype=np.float32)
    mask = np.asarray(mask, dtype=np.float32)

    w_eff = _w_eff(weight, mask)
    if "nc" not in _CACHE:
        _CACHE["nc"] = _build_bass()
    nc = _CACHE["nc"]

    xt_full = np.ascontiguousarray(x.T)  # [128, BATCH]
    in_maps = [
        {
            "xt": np.ascontiguousarray(xt_full[:, c * SHARD:(c + 1) * SHARD]),
            "w": w_eff,
        }
        for c in range(N_CORES)
    ]
    res = run_bass_kernel_spmd(nc, in_maps, core_ids=list(range(N_CORES)))
    zt = np.concatenate([r["zt"] for r in res.results], axis=1)  # [128, BATCH]
    return np.ascontiguousarray(zt.T)
